# revision 11
# baseline (speedup 1.0000x reference)
"""Trainium2 Bass kernel for MQA sparse attention (nn_Attention_83356725281353).

Strategy: batch-parallel across 8 NeuronCores (4 batches each). Host-side
staging does pure layout work (roll of the KV cache, transposes, 1/sqrt(d)
folded into wq) so the device kernel is a clean stream:

  per core:
    q/k_new/v_new projections (f32r matmuls, wq streamed from HBM)
    per batch-pair (2 batches stacked on 128 partitions):
      p = qT_b @ kT-chunk     (f32r, N=512)          [rows=(h,q), kv]
      e = exp(p + bias)       (DVE add + ACT exp, row-sums via accum_out)
      aT = transpose(e)       (TensorE, bf16)
      o += aT.T @ v           (bf16, accumulated in PSUM over kv)
    o /= rowsums; out = oT.T @ wo + bo  (f32r, wo streamed)

Self-contained: hardcodes all shapes; builds/compiles the Bass graph once per
process and runs it via run_bass_kernel_spmd on cores 0-7.
"""

import numpy as np

B, Q, DIM, H, HD, KV = 32, 4, 2048, 16, 128, 8192
NCORES = 8
BPC = B // NCORES            # 4 batches per core
BQ = BPC * Q                 # 16 (b,q) rows per core
ROWS = H * Q                 # 64 attention rows per batch
NPAIR = BPC // 2             # 2 batch-pairs per core
DT = 16                      # dim tiles (DIM/128)
KCH = 2048                   # kv chunk width
NCH = KV // KCH              # 4 chunks per batch

_CACHE = {}


def _build():
    import concourse.bass as bass
    import concourse.tile as tile
    from concourse import bacc, mybir, masks

    f32 = mybir.dt.float32
    f32r = mybir.dt.float32r
    bf16 = mybir.dt.bfloat16

    nc = bacc.Bacc("TRN2", target_bir_lowering=False, debug=False,
                   num_devices=NCORES)

    # Tensors that feed f32r matmuls are declared float32r end-to-end (same
    # bits as f32; walrus requires the producer chain to be f32r-rounded).
    xT = nc.dram_tensor("xT", [DIM, BQ], f32r, kind="ExternalInput").ap()
    wq = nc.dram_tensor("wq", [DIM, H * HD], f32r, kind="ExternalInput").ap()
    bq = nc.dram_tensor("bq", [1, H * HD], f32r, kind="ExternalInput").ap()
    wk = nc.dram_tensor("wk", [DIM, HD], f32r, kind="ExternalInput").ap()
    bk = nc.dram_tensor("bk", [1, HD], f32r, kind="ExternalInput").ap()
    wv = nc.dram_tensor("wv", [DIM, HD], f32r, kind="ExternalInput").ap()
    bv = nc.dram_tensor("bv", [1, HD], f32r, kind="ExternalInput").ap()
    kT = nc.dram_tensor("kT", [BPC, HD, KV], f32r, kind="ExternalInput").ap()
    vv = nc.dram_tensor("vv", [BPC, KV, HD], f32, kind="ExternalInput").ap()
    bias = nc.dram_tensor("bias", [NPAIR, 2 * ROWS, KV], f32,
                          kind="ExternalInput").ap()
    wo = nc.dram_tensor("wo", [H * HD, DIM], f32r, kind="ExternalInput").ap()
    bo = nc.dram_tensor("bo", [1, DIM], f32r, kind="ExternalInput").ap()
    ones = nc.dram_tensor("ones", [1, BQ], f32r, kind="ExternalInput").ap()
    zeros = nc.dram_tensor("zeros", [128, 2 * 128], f32r,
                           kind="ExternalInput").ap()
    out = nc.dram_tensor("out", [BQ, DIM], f32, kind="ExternalOutput").ap()

    with tile.TileContext(nc) as tc:
        _body(tc, nc, bass, mybir, masks, xT, wq, bq, wk, bk, wv, bv, kT, vv,
              bias, wo, bo, ones, zeros, out)

    nc.compile()
    return nc


def _body(tc, nc, bass, mybir, masks, xT, wq, bq, wk, bk, wv, bv, kT, vv,
          bias, wo, bo, ones, zeros, out):
    from contextlib import ExitStack

    f32 = mybir.dt.float32
    f32r = mybir.dt.float32r
    bf16 = mybir.dt.bfloat16
    EXP = mybir.ActivationFunctionType.Exp

    with ExitStack() as octx:
        const = octx.enter_context(tc.tile_pool(name="const", bufs=1))
        wpool = octx.enter_context(tc.tile_pool(name="w", bufs=3))
        kpool = octx.enter_context(tc.tile_pool(name="kt", bufs=4))
        vpool = octx.enter_context(tc.tile_pool(name="vt", bufs=4))
        bpool = octx.enter_context(tc.tile_pool(name="bias", bufs=3))
        apool = octx.enter_context(tc.tile_pool(name="a", bufs=3))

        ident_f = const.tile([128, 128], f32, tag="idf")
        ident_b = const.tile([128, 128], bf16, tag="idb")
        masks.make_identity(nc, ident_f[:])
        masks.make_identity(nc, ident_b[:])
        ones16 = const.tile([1, BQ], f32r, tag="ones16")
        nc.sync.dma_start(ones16[:], ones)

        xT_sb = const.tile([128, DT * BQ], f32r, tag="xT")
        nc.sync.dma_start(xT_sb[:].rearrange("p (t m) -> p t m", t=DT),
                          xT.rearrange("(t p) m -> p t m", p=128))
        wk_sb = const.tile([128, DT * HD], f32r, tag="wk")
        nc.sync.dma_start(wk_sb[:].rearrange("p (t e) -> p t e", t=DT),
                          wk.rearrange("(t p) e -> p t e", p=128))
        wv_sb = const.tile([128, DT * HD], f32r, tag="wv")
        nc.sync.dma_start(wv_sb[:].rearrange("p (t e) -> p t e", t=DT),
                          wv.rearrange("(t p) e -> p t e", p=128))
        bq_sb = const.tile([1, H * HD], f32r, tag="bq")
        nc.sync.dma_start(bq_sb[:], bq)
        bk_sb = const.tile([1, HD], f32r, tag="bk")
        nc.sync.dma_start(bk_sb[:], bk)
        bv_sb = const.tile([1, HD], f32r, tag="bv")
        nc.sync.dma_start(bv_sb[:], bv)
        bo_sb = const.tile([1, DIM], f32r, tag="bo")
        nc.sync.dma_start(bo_sb[:], bo)

        q_sb = const.tile([BQ, H * HD], f32, tag="q")
        kn_sb = const.tile([BQ, HD], f32, tag="kn")
        vn_sb = const.tile([BQ, HD], f32, tag="vn")
        # qT: per pair j a 256-col block [qb0(h,q) | 0*64 | 0*64 | qb1(h,q)]
        # so both p-matmuls run at tile_position (0,0) with M=128 (f32r
        # cannot col-tile at partition offset 64).
        qT_sb = const.tile([128, NPAIR * 2 * 128], f32r, tag="qT")
        for j in range(NPAIR):
            nc.sync.dma_start(qT_sb[:, j * 256:(j + 1) * 256], zeros)
        knT_sb = const.tile([128, BQ], f32r, tag="knT")
        # oT layout: [e=128, (h,b,q)] col = h*16 + b*4 + q
        oT_sb = const.tile([128, BPC * ROWS], f32r, tag="oT")
        NSUB = 4 * NCH                        # 512-wide sub-chunks per batch
        sums = const.tile([128, 2 * (NSUB + 1)], f32, tag="sums")

        # ---------------- Phase P: projections -----------------------------
        with (tc.tile_pool(name="qps", bufs=1, space="PSUM") as qps,
              tc.tile_pool(name="ptr", bufs=1, space="PSUM") as ptr):
            q_ps = qps.tile([BQ, H * HD], f32, tag="qacc")
            kv_ps = qps.tile([BQ, 2 * HD], f32, tag="kvacc")
            for t in range(DT):
                w_t = wpool.tile([128, H * HD], f32r, tag="wtile")
                nc.sync.dma_start(w_t[:], wq[t * 128:(t + 1) * 128, :])
                lhs = xT_sb[:, t * BQ:(t + 1) * BQ]
                for n in range(4):
                    nc.tensor.matmul(q_ps[:, n * 512:(n + 1) * 512], lhs,
                                     w_t[:, n * 512:(n + 1) * 512],
                                     start=(t == 0), stop=False)
                nc.tensor.matmul(kv_ps[:, 0:HD], lhs,
                                 wk_sb[:, t * HD:(t + 1) * HD],
                                 start=(t == 0), stop=False)
                nc.tensor.matmul(kv_ps[:, HD:2 * HD], lhs,
                                 wv_sb[:, t * HD:(t + 1) * HD],
                                 start=(t == 0), stop=False)
            # bias rows via ones-row matmul (K=1)
            ones_r = ones16[0:1, :]
            for n in range(4):
                nc.tensor.matmul(q_ps[:, n * 512:(n + 1) * 512], ones_r,
                                 bq_sb[0:1, n * 512:(n + 1) * 512],
                                 start=False, stop=True)
            nc.tensor.matmul(kv_ps[:, 0:HD], ones_r, bk_sb[0:1, :],
                             start=False, stop=True)
            nc.tensor.matmul(kv_ps[:, HD:2 * HD], ones_r,
                             bv_sb[0:1, :], start=False, stop=True)

            nc.vector.tensor_copy(q_sb[:], q_ps[:])
            nc.vector.tensor_copy(kn_sb[:], kv_ps[:, 0:HD])
            nc.vector.tensor_copy(vn_sb[:], kv_ps[:, HD:2 * HD])

            # transpose q: per head [16,128] -> [128,16] into one PSUM
            # tile laid out (h,b,q); then one strided copy per batch into
            # the padded qT blocks.
            qtr = ptr.tile([128, H * BQ], f32, tag="qtr")
            for h in range(H):
                nc.tensor.transpose(qtr[:, h * BQ:(h + 1) * BQ],
                                    q_sb[:, h * HD:(h + 1) * HD],
                                    ident_f[0:BQ, 0:BQ])
            qtr_hbq = qtr[:].rearrange("p (h b q) -> p h b q", h=H, b=BPC)
            for b in range(BPC):
                off = (b // 2) * 256 + (b % 2) * 192
                dst = qT_sb[:, off:off + ROWS].rearrange(
                    "p (h q) -> p h q", h=H)
                nc.vector.tensor_copy(dst, qtr_hbq[:, :, b, :])
            trk = ptr.tile([128, BQ], f32, tag="tr")
            nc.tensor.transpose(trk[:], kn_sb[:], ident_f[0:BQ, 0:BQ])
            nc.vector.tensor_copy(knT_sb[:], trk[:])

        # ---------------- Phase A: attention, per batch-pair ---------------
        with (tc.tile_pool(name="pps", bufs=2, space="PSUM") as pps,
              tc.tile_pool(name="tps", bufs=2, space="PSUM") as tps,
              tc.tile_pool(name="ops", bufs=2, space="PSUM") as ops):
            for j in range(NPAIR):
                b0, b1 = 2 * j, 2 * j + 1
                o_ps = ops.tile([128, HD], f32, tag="o")
                for c in range(NCH):
                    kt0 = kpool.tile([128, KCH], f32r, tag="kt")
                    nc.sync.dma_start(kt0[:], kT[b0][:, c * KCH:(c + 1) * KCH])
                    kt1 = kpool.tile([128, KCH], f32r, tag="kt")
                    nc.sync.dma_start(kt1[:], kT[b1][:, c * KCH:(c + 1) * KCH])
                    v0 = vpool.tile([128, KCH], bf16, tag="vt")
                    nc.gpsimd.dma_start(
                        v0[:].rearrange("p (n e) -> p n e", n=16),
                        vv[b0].rearrange("(n p) e -> p n e", p=128)
                          [:, c * 16:(c + 1) * 16, :])
                    v1 = vpool.tile([128, KCH], bf16, tag="vt")
                    nc.gpsimd.dma_start(
                        v1[:].rearrange("p (n e) -> p n e", n=16),
                        vv[b1].rearrange("(n p) e -> p n e", p=128)
                          [:, c * 16:(c + 1) * 16, :])
                    bias_sb = bpool.tile([128, KCH], f32, tag="bias")
                    nc.sync.dma_start(bias_sb[:],
                                      bias[j][:, c * KCH:(c + 1) * KCH])
                    if c == NCH - 1:
                        # roll-in: overwrite last 4 kv positions with new k/v
                        nc.vector.tensor_copy(kt0[:, KCH - 4:KCH],
                                              knT_sb[:, b0 * 4:b0 * 4 + 4])
                        nc.vector.tensor_copy(kt1[:, KCH - 4:KCH],
                                              knT_sb[:, b1 * 4:b1 * 4 + 4])
                        # DMA: arbitrary partition offsets + f32->bf16 cast
                        nc.gpsimd.dma_start(v0[124:128, 15 * HD:16 * HD],
                                            vn_sb[b0 * 4:b0 * 4 + 4, :])
                        nc.gpsimd.dma_start(v1[124:128, 15 * HD:16 * HD],
                                            vn_sb[b1 * 4:b1 * 4 + 4, :])
                    for n in range(4):
                        p_ps = pps.tile([128, 512], f32, tag="p")
                        nc.tensor.matmul(
                            p_ps[:, :],
                            qT_sb[:, j * 256:j * 256 + 128],
                            kt0[:, n * 512:(n + 1) * 512],
                            start=True, stop=False)
                        nc.tensor.matmul(
                            p_ps[:, :],
                            qT_sb[:, j * 256 + 128:j * 256 + 256],
                            kt1[:, n * 512:(n + 1) * 512],
                            start=False, stop=True)
                        e_sb = apool.tile([128, 512], f32, tag="e")
                        nc.vector.tensor_tensor(
                            e_sb[:], p_ps[:], bias_sb[:, n * 512:(n + 1) * 512],
                            op=mybir.AluOpType.add)
                        a_bf = apool.tile([128, 512], bf16, tag="abf")
                        scol = j * (4 * NCH + 1) + c * 4 + n
                        nc.scalar.activation(a_bf[:], e_sb[:], EXP,
                                             accum_out=sums[:, scol:scol + 1])
                        for piece in range(4):
                            tr = tps.tile([128, 128], bf16, tag="tr")
                            nc.tensor.transpose(
                                tr[:], a_bf[:, piece * 128:(piece + 1) * 128],
                                ident_b[:])
                            aT = apool.tile([128, 128], bf16, tag="aT")
                            nc.vector.tensor_copy(aT[:], tr[:])
                            kvt = c * 16 + n * 4 + piece
                            first, last = (kvt == 0), (kvt == 63)
                            nc.tensor.matmul(
                                o_ps[0:ROWS, :], aT[:, 0:ROWS],
                                v0[:, (n * 4 + piece) * HD:
                                   (n * 4 + piece + 1) * HD],
                                start=first, stop=last)
                            nc.tensor.matmul(
                                o_ps[ROWS:128, :], aT[:, ROWS:128],
                                v1[:, (n * 4 + piece) * HD:
                                   (n * 4 + piece + 1) * HD],
                                start=first, stop=last, tile_position=(0, 64))
                # softmax denominators and normalization for the pair
                _finalize_pair(tc, nc, mybir, apool, tps, j, o_ps, sums,
                               oT_sb, ident_f)

        # ---------------- Phase O: output projection ------------------------
        with tc.tile_pool(name="outps", bufs=1, space="PSUM") as outps:
            out_ps = outps.tile([BQ, DIM], f32, tag="out")
            for h in range(H):
                w_t = wpool.tile([128, DIM], f32r, tag="wtile")
                nc.sync.dma_start(w_t[:], wo[h * HD:(h + 1) * HD, :])
                lhs = oT_sb[:, h * BQ:(h + 1) * BQ]
                for n in range(4):
                    nc.tensor.matmul(out_ps[:, n * 512:(n + 1) * 512], lhs,
                                     w_t[:, n * 512:(n + 1) * 512],
                                     start=(h == 0), stop=False)
            ones_r = ones16[0:1, :]
            for n in range(4):
                nc.tensor.matmul(out_ps[:, n * 512:(n + 1) * 512], ones_r,
                                 bo_sb[0:1, n * 512:(n + 1) * 512],
                                 start=False, stop=True)
            out_sb = const.tile([BQ, DIM], f32, tag="osb")
            nc.vector.tensor_copy(out_sb[:], out_ps[:])
            nc.sync.dma_start(out, out_sb[:])


def _finalize_pair(tc, nc, mybir, apool, tps, j, o_ps, sums, oT_sb, ident_f):
    f32 = mybir.dt.float32
    NSUB = 4 * NCH
    base = j * (NSUB + 1)
    tot = sums[:, base + NSUB:base + NSUB + 1]
    nc.vector.reduce_sum(tot, sums[:, base:base + NSUB],
                         axis=mybir.AxisListType.X)
    recip = apool.tile([128, 1], f32, tag="recip")
    nc.vector.reciprocal(recip[:], tot)
    o_sb = apool.tile([128, HD], f32, tag="osb")
    nc.vector.tensor_scalar_mul(o_sb[:], o_ps[:], recip[:])
    tr = tps.tile([128, 128], f32, tag="tr")
    nc.tensor.transpose(tr[:], o_sb[:], ident_f[:])
    oT_4d = oT_sb[:].rearrange("p (h b q) -> p h b q", h=H, b=BPC)
    for b2 in range(2):
        nc.vector.tensor_copy(
            oT_4d[:, :, 2 * j + b2, :],
            tr[:, b2 * ROWS:(b2 + 1) * ROWS].rearrange(
                "p (h q) -> p h q", h=H))


def _get_nc():
    if "nc" not in _CACHE:
        _CACHE["nc"] = _build()
    return _CACHE["nc"]


def kernel(x, attn_bias, cache_k, cache_v, wq, bq, wk, bk, wv, bv, wo, bo):
    from concourse.bass_utils import run_bass_kernel_spmd

    nc = _get_nc()
    scale = np.float32(1.0 / np.sqrt(HD))

    x = np.asarray(x, np.float32)
    xT_full = np.ascontiguousarray(x.reshape(B * Q, DIM).T)          # [DIM, 128]
    wq2 = np.ascontiguousarray(
        (np.asarray(wq, np.float32) * scale).reshape(DIM, H * HD))
    bq2 = np.ascontiguousarray(
        (np.asarray(bq, np.float32) * scale).reshape(1, H * HD))
    wk2 = np.ascontiguousarray(np.asarray(wk, np.float32))
    bk2 = np.asarray(bk, np.float32).reshape(1, HD)
    wv2 = np.ascontiguousarray(np.asarray(wv, np.float32))
    bv2 = np.asarray(bv, np.float32).reshape(1, HD)
    kTh = np.ascontiguousarray(
        np.roll(np.asarray(cache_k, np.float32), -Q, axis=1).transpose(0, 2, 1))
    vrh = np.ascontiguousarray(
        np.roll(np.asarray(cache_v, np.float32), -Q, axis=1))
    biasP = np.ascontiguousarray(
        np.asarray(attn_bias, np.float32).reshape(B // 2, 2 * ROWS, KV))
    wo2 = np.ascontiguousarray(np.asarray(wo, np.float32).reshape(H * HD, DIM))
    bo2 = np.asarray(bo, np.float32).reshape(1, DIM)

    in_maps = []
    for c in range(NCORES):
        in_maps.append({
            "xT": np.ascontiguousarray(xT_full[:, c * BQ:(c + 1) * BQ]),
            "wq": wq2, "bq": bq2, "wk": wk2, "bk": bk2, "wv": wv2, "bv": bv2,
            "kT": kTh[c * BPC:(c + 1) * BPC],
            "vv": vrh[c * BPC:(c + 1) * BPC],
            "bias": biasP[NPAIR * c:NPAIR * (c + 1)],
            "wo": wo2, "bo": bo2,
            "ones": np.ones((1, BQ), np.float32),
            "zeros": np.zeros((128, 2 * 128), np.float32),
        })

    res = run_bass_kernel_spmd(nc, in_maps, core_ids=list(range(NCORES)))
    _CACHE["last_result"] = res
    outs = [res.results[c]["out"] for c in range(NCORES)]
    return np.concatenate(outs, axis=0).reshape(B, Q, DIM).astype(np.float32)
